# revision 2
# baseline (speedup 1.0000x reference)
"""Trainium2 Bass kernel v2 for nn_DiffusionLoss (retrieval_knn), 8-core SPMD.

Core = 2*batch + half; each core owns 4096 rows (i) of its batch and all 8192
columns (j, own-first permuted order).

Numerics: all big matmuls use an exact bf16 hi/lo (Dekker) decomposition of
the coordinates, carried in two host-packed tensors:
  sjt [13, 8192] bf16  rows [xh*3, xh*3, xl*3, -x2h, -x2l, 1, 1]
  mbt [13, 4096] bf16  rows [2xh*3, 2xl*3, 2xh*3, 1, 1, th, tl]
fwd  G'[i,j] = lhsT=mbt[0:11, i-slice] x rhs=sjt[0:11, j-chunk]
bwd  H[j,i]  = lhsT=sjt[:, jt] x rhs=mbt[:, i-group]  (tau rows device-filled)
The per-k addends of fwd G'[i,j] and the G' part of bwd H[j,i] are bitwise
identical, so the mask compare H >= 0 <=> G' >= tau-EPS is exact to ~1 ulp.

Per strip, tau = 8th-NN G' value = 9th largest incl. self, extracted with
max8 on PSUM chunks (no diag masking: self is the row max by construction)
+ merge max8 + match_replace + max8.

Mask path: ACT Sigmoid(1e6*H) -> fp16 {0,1}; mask-mm fp16 against wtab
(x,y,z,x2,1) accumulates (s1,s2,cnt) in PSUM; self subtracted exactly.

The emission interleaves bwd/mask of group g-1 with fwd/max8 of group g so
PE, ACT and DVE all stay busy.
"""
import numpy as np
import ml_dtypes

BF16 = ml_dtypes.bfloat16

B, N, HALF = 4, 8192, 4096
NSTRIP = 32          # 128-row strips per core
NCH = 8              # fwd chunks of 1024 per strip
NJT = 64             # bwd j-tiles of 128
NGROUP = 8           # i-groups of 512
EPS = 3e-5
SIG_SCALE = 1.0e6
NEG_BIG = -1.0e30

_COMPILED = None
REPS_TIMING = 4


def _build_core(reps=1):
    import concourse.bass as bass
    import concourse.mybir as mybir
    import concourse.tile as tile

    f32 = mybir.dt.float32
    f16 = mybir.dt.float16
    bf16 = mybir.dt.bfloat16
    AF = mybir.ActivationFunctionType
    ALU = mybir.AluOpType
    AX = mybir.AxisListType

    nc = bass.Bass()
    sjt = nc.dram_tensor("sjt", [13, N], bf16, kind="ExternalInput")
    mbt = nc.dram_tensor("mbt", [13, HALF], bf16, kind="ExternalInput")
    wtab = nc.dram_tensor("wtab", [128, NJT * 5], f16, kind="ExternalInput")
    prow = nc.dram_tensor("prow", [128, 96], f32, kind="ExternalInput")
    trow = nc.dram_tensor("trow", [128, 96], f32, kind="ExternalInput")
    pfeat = nc.dram_tensor("pfeat", [128, 8], f32, kind="ExternalInput")
    tfeat = nc.dram_tensor("tfeat", [128, 8], f32, kind="ExternalInput")
    bsta = nc.dram_tensor("bsta", [21, 512], bf16, kind="ExternalInput")
    bmov = nc.dram_tensor("bmov", [21, HALF], bf16, kind="ExternalInput")
    c1r2 = nc.dram_tensor("c1r2", [128, 4], f32, kind="ExternalInput")
    out = nc.dram_tensor("out", [128, 8], f32, kind="ExternalOutput")

    with tile.TileContext(nc) as tc:
        with tc.tile_pool(name="persist", bufs=1) as pp, \
             tc.tile_pool(name="mask", bufs=4) as mp, \
             tc.tile_pool(name="small", bufs=1) as smp, \
             tc.tile_pool(name="psF", bufs=2, space="PSUM") as psF, \
             tc.tile_pool(name="psB", bufs=3, space="PSUM") as psB, \
             tc.tile_pool(name="psS", bufs=1, space="PSUM") as psS:

            for _rep in range(reps):
                # ---------------- persistent loads ----------------
                t_sjt = pp.tile([13, N], bf16)
                nc.sync.dma_start(t_sjt[:], sjt[:])
                t_mbt = pp.tile([13, HALF], bf16)
                nc.sync.dma_start(t_mbt[:], mbt[:])
                t_wtab = pp.tile([128, NJT * 5], f16)
                nc.sync.dma_start(t_wtab[:], wtab[:])
                wtab5 = t_wtab[:].rearrange("p (j c) -> p j c", c=5)
                t_bsta = pp.tile([21, 512], bf16)
                nc.sync.dma_start(t_bsta[:], bsta[:])
                t_bmov = pp.tile([21, HALF], bf16)
                nc.sync.dma_start(t_bmov[:], bmov[:])
                t_c1r2 = pp.tile([128, 4], f32)
                nc.sync.dma_start(t_c1r2[:], c1r2[:])

                fill_zero = nc.gpsimd.to_reg(0.0)
                eye128 = pp.tile([128, 128], f32)
                nc.gpsimd.memset(eye128[:], 1.0)
                nc.gpsimd.affine_select(eye128[:], eye128[:], [[-1, 128]],
                                        ALU.is_equal, fill_zero,
                                        channel_multiplier=1)
                eye5 = pp.tile([5, 5], f32)
                nc.gpsimd.memset(eye5[:], 1.0)
                nc.gpsimd.affine_select(eye5[:], eye5[:], [[-1, 5]],
                                        ALU.is_equal, fill_zero,
                                        channel_multiplier=1)

                SaTT = pp.tile([128, 32, 5], f32)
                saRow = pp.tile([5, NGROUP * 512], f32)

                # ---------------- software-pipelined main loop ---------
                # block gb: fwd strips of group gb (gb<8) interleaved with
                # bwd+maskmm of group gb-1 (gb>0)
                for gb in range(NGROUP + 1):
                    do_fwd = gb < NGROUP
                    do_bwd = gb > 0
                    gB = gb - 1   # bwd group index
                    if do_fwd:
                        cand = smp.tile([128, 256], f32, tag="cand")
                        tau4 = smp.tile([128, 4], f32, tag="tau4")
                    pS = None
                    for step in range(32):
                        # -- bwd pair of j-tiles + mask + mask-mm --
                        if do_bwd:
                            for q in range(2):
                                jt = step * 2 + q
                                pB = psB.tile([128, 512], f32, tag="pB")
                                nc.tensor.matmul(
                                    pB[:],
                                    t_sjt[:, jt * 128:(jt + 1) * 128],
                                    t_mbt[:, gB * 512:(gB + 1) * 512],
                                    start=True, stop=True)
                                mt = mp.tile([128, 512], f16, tag="mt")
                                nc.scalar.activation(mt[:], pB[:], AF.Sigmoid,
                                                     scale=SIG_SCALE)
                                if jt == 0:
                                    pS = psS.tile([5, 512], f32, tag="pS")
                                nc.tensor.matmul(
                                    pS[:],
                                    t_wtab[:, jt * 5:(jt + 1) * 5],
                                    mt[:],
                                    start=(jt == 0), stop=(jt == NJT - 1),
                                    skip_group_check=(jt != 0))
                        # -- fwd chunk + max8 --
                        if do_fwd:
                            s4 = step // 8          # strip within group
                            c = step % 8            # chunk within strip
                            s = gb * 4 + s4
                            pF = psF.tile([128, 1024], f32, tag="pF")
                            for q in range(2):
                                nc.tensor.matmul(
                                    pF[:, q * 512:(q + 1) * 512],
                                    t_mbt[0:11, s * 128:(s + 1) * 128],
                                    t_sjt[0:11,
                                          (c * 1024 + q * 512):
                                          (c * 1024 + (q + 1) * 512)],
                                    start=True, stop=True)
                            nc.vector.max(
                                cand[:, s4 * 64 + c * 8:s4 * 64 + c * 8 + 8],
                                pF[:])
                            if c == 7:
                                # strip done: tau = 9th largest incl self
                                top8a = smp.tile([128, 8], f32, tag="top8a")
                                nc.vector.max(top8a[:],
                                              cand[:, s4 * 64:s4 * 64 + 64])
                                mr = smp.tile([128, 64], f32, tag="mr")
                                nc.vector.match_replace(
                                    mr[:], top8a[:],
                                    cand[:, s4 * 64:s4 * 64 + 64], NEG_BIG)
                                top9 = smp.tile([128, 8], f32, tag="top9")
                                nc.vector.max(top9[:], mr[:])
                                nc.vector.tensor_copy(tau4[:, s4:s4 + 1],
                                                      top9[:, 0:1])
                    # -- end of block: group bookkeeping --
                    if do_bwd:
                        # drain mask-mm accumulator
                        nc.scalar.copy(saRow[:, gB * 512:(gB + 1) * 512],
                                       pS[:])
                    if do_fwd:
                        # tau -> row layout, EPS-shift, bf16 hi/lo split
                        pT = psB.tile([128, 512], f32, tag="pB")
                        nc.tensor.transpose(pT[0:4, 0:128], tau4[:],
                                            eye128[:])
                        trow4 = smp.tile([4, 128], f32, tag="trow4")
                        nc.vector.tensor_scalar(trow4[:], pT[0:4, 0:128],
                                                -1.0, EPS,
                                                op0=ALU.mult, op1=ALU.add)
                        th4 = smp.tile([4, 128], bf16, tag="th4")
                        nc.vector.tensor_copy(th4[:], trow4[:])
                        tl4 = smp.tile([4, 128], bf16, tag="tl4")
                        nc.vector.tensor_tensor(tl4[:], trow4[:], th4[:],
                                                op=ALU.subtract)
                        nc.sync.dma_start(
                            t_mbt[11:12, gb * 512:(gb + 1) * 512], th4[:])
                        nc.sync.dma_start(
                            t_mbt[12:13, gb * 512:(gb + 1) * 512], tl4[:])

                # ---------------- continuity finalize ----------------
                # SaTT[p, w, :] = (s1x, s1y, s1z, s2, cnt) for i = w*128+p
                for w in range(32):
                    ptW = psB.tile([128, 512], f32, tag="pB")
                    nc.tensor.transpose(
                        ptW[0:128, 0:5],
                        saRow[:, w * 128:(w + 1) * 128],
                        eye5[:])
                    nc.scalar.copy(SaTT[:, w, :], ptW[0:128, 0:5])

                # subtract self (exact fp16 table values), then variance
                Wself = pp.tile([128, 32, 4], f32)
                nc.vector.tensor_copy(Wself[:], wtab5[:, 0:32, 0:4])
                nc.vector.tensor_tensor(SaTT[:, :, 0:4], SaTT[:, :, 0:4],
                                        Wself[:], op=ALU.subtract)
                nc.vector.tensor_scalar_add(SaTT[:, :, 4], SaTT[:, :, 4],
                                            -1.0)
                fzT = pp.tile([128, 32, 4], f32)
                nc.vector.tensor_tensor(fzT[:, :, 0:3], SaTT[:, :, 0:3],
                                        SaTT[:, :, 0:3], op=ALU.mult)
                nc.vector.tensor_add(fzT[:, :, 0], fzT[:, :, 0],
                                     fzT[:, :, 1])
                nc.vector.tensor_add(fzT[:, :, 0], fzT[:, :, 0],
                                     fzT[:, :, 2])
                nc.vector.reciprocal(fzT[:, :, 1], SaTT[:, :, 4])
                nc.vector.tensor_tensor(fzT[:, :, 2], fzT[:, :, 0],
                                        fzT[:, :, 1], op=ALU.mult)
                nc.vector.tensor_tensor(fzT[:, :, 2], fzT[:, :, 2],
                                        fzT[:, :, 1], op=ALU.mult)
                nc.vector.tensor_tensor(fzT[:, :, 3], SaTT[:, :, 3],
                                        fzT[:, :, 1], op=ALU.mult)
                nc.vector.tensor_sub(fzT[:, :, 3], fzT[:, :, 3],
                                     fzT[:, :, 2])
                nc.vector.tensor_scalar_mul(fzT[:, :, 3], fzT[:, :, 3], 8.0)
                cont_p = pp.tile([128, 1], f32)
                nc.vector.tensor_reduce(cont_p[:], fzT[:, :, 3], axis=AX.X,
                                        op=ALU.add)

                # ---------------- recon / percep ----------------
                t_prow = smp.tile([128, 96], f32, tag="pr")
                nc.sync.dma_start(t_prow[:], prow[:])
                t_trow = smp.tile([128, 96], f32, tag="tr")
                nc.sync.dma_start(t_trow[:], trow[:])
                dif = smp.tile([128, 96], f32, tag="dif")
                nc.vector.tensor_sub(dif[:], t_prow[:], t_trow[:])
                rsc = smp.tile([128, 96], f32, tag="rsc")
                rec_acc = pp.tile([128, 1], f32)
                nc.scalar.activation(rsc[:], dif[:], AF.Square,
                                     accum_out=rec_acc[:])
                t_pf = smp.tile([128, 8], f32, tag="pf")
                nc.sync.dma_start(t_pf[:], pfeat[:])
                t_tf = smp.tile([128, 8], f32, tag="tf")
                nc.sync.dma_start(t_tf[:], tfeat[:])
                dff = smp.tile([128, 8], f32, tag="dff2")
                nc.vector.tensor_sub(dff[:], t_pf[:], t_tf[:])
                fsc = smp.tile([128, 8], f32, tag="fsc")
                per_acc = pp.tile([128, 1], f32)
                nc.scalar.activation(fsc[:], dff[:], AF.Square,
                                     accum_out=per_acc[:])

                # ---------------- boundary ----------------
                rm = pp.tile([128, 16], f32)
                for st in range(4):
                    for cb in range(4):
                        pC = psF.tile([128, 1024], f32, tag="pF")
                        for q in range(2):
                            nc.tensor.matmul(
                                pC[:, q * 512:(q + 1) * 512],
                                t_bsta[:, st * 128:(st + 1) * 128],
                                t_bmov[:, (cb * 1024 + q * 512):
                                       (cb * 1024 + (q + 1) * 512)],
                                start=True, stop=True)
                        nc.vector.tensor_reduce(
                            rm[:, st * 4 + cb:st * 4 + cb + 1],
                            pC[:], axis=AX.X, op=ALU.max)
                gmax = pp.tile([128, 4], f32)
                nc.vector.tensor_reduce(
                    gmax[:], rm[:].rearrange("p (a b) -> p a b", b=4),
                    axis=AX.X, op=ALU.max)
                d2 = pp.tile([128, 4], f32)
                nc.vector.tensor_sub(d2[:], t_c1r2[:], gmax[:])
                nc.vector.tensor_scalar_max(d2[:], d2[:], 0.0)
                dd = pp.tile([128, 4], f32)
                nc.scalar.activation(dd[:], d2[:], AF.Sqrt)
                bm = pp.tile([128, 4], f32)
                nc.vector.tensor_scalar(bm[:], dd[:], 0.1, None,
                                        op0=ALU.is_lt)
                dm = pp.tile([128, 4], f32)
                nc.vector.tensor_tensor(dm[:], dd[:], bm[:], op=ALU.mult)
                bsum = pp.tile([128, 1], f32)
                nc.vector.tensor_reduce(bsum[:], dm[:], axis=AX.X,
                                        op=ALU.add)
                bcnt = pp.tile([128, 1], f32)
                nc.vector.tensor_reduce(bcnt[:], bm[:], axis=AX.X,
                                        op=ALU.add)

                # ---------------- output ----------------
                o = pp.tile([128, 8], f32)
                nc.vector.memset(o[:], 0.0)
                nc.vector.tensor_copy(o[:, 0:1], cont_p[:])
                nc.vector.tensor_copy(o[:, 1:2], rec_acc[:])
                nc.vector.tensor_copy(o[:, 2:3], per_acc[:])
                nc.vector.tensor_copy(o[:, 3:4], bsum[:])
                nc.vector.tensor_copy(o[:, 4:5], bcnt[:])
                nc.sync.dma_start(out[:], o[:])
    return nc


def _split_excess_waits(nc, mybir, max_waits=1):
    for fn in nc.m.functions:
        for bb in fn.blocks:
            new_insts = []
            for inst in bb.instructions:
                si = getattr(inst, 'sync_info', None)
                if si is not None and si.on_wait and len(si.on_wait) > max_waits:
                    waits = list(si.on_wait)
                    rest, keep = waits[:-max_waits], waits[-max_waits:]
                    for i in range(0, len(rest), max_waits):
                        nop = mybir.InstNoOp(name=f"{inst.name}-ws{i}")
                        nop.engine = inst.engine
                        nop.sync_info = mybir.SyncInfo(
                            on_wait=rest[i:i + max_waits], on_update=[])
                        new_insts.append(nop)
                    inst.sync_info = mybir.SyncInfo(
                        on_wait=keep,
                        on_update=list(si.on_update) if si.on_update else [])
                new_insts.append(inst)
            bb.instructions = new_insts


class _Compiled:
    def __init__(self, reps=1):
        import jax
        import concourse.mybir as mybir
        from concourse import bass2jax
        from jax.sharding import Mesh, PartitionSpec
        from jax.experimental.shard_map import shard_map

        nc = _build_core(reps)
        _split_excess_waits(nc, mybir)
        bass2jax.install_neuronx_cc_hook()
        partition_name = (nc.partition_id_tensor.name
                          if nc.partition_id_tensor else None)
        in_names, out_names, out_avals = [], [], []
        for alloc in nc.m.functions[0].allocations:
            if not isinstance(alloc, mybir.MemoryLocationSet):
                continue
            name = alloc.memorylocations[0].name
            if alloc.kind == "ExternalInput":
                if name != partition_name:
                    in_names.append(name)
            elif alloc.kind == "ExternalOutput":
                out_names.append(name)
                out_avals.append(jax.core.ShapedArray(
                    tuple(alloc.tensor_shape), mybir.dt.np(alloc.dtype)))
        self.in_names, self.out_names, self.out_avals = \
            in_names, out_names, out_avals
        in_names_all = in_names + out_names
        if partition_name:
            in_names_all.append(partition_name)

        def _body(*args):
            operands = list(args)
            if partition_name is not None:
                operands.append(bass2jax.partition_id_tensor())
            return tuple(bass2jax._bass_exec_p.bind(
                *operands, out_avals=tuple(out_avals),
                in_names=tuple(in_names_all), out_names=tuple(out_names),
                lowering_input_output_aliases=(), sim_require_finite=True,
                sim_require_nnan=True, nc=nc))

        devices = jax.devices()[:8]
        mesh = Mesh(np.asarray(devices), ("core",))
        n_in = len(in_names) + len(out_names)
        self.fn = jax.jit(
            shard_map(_body, mesh=mesh,
                      in_specs=(PartitionSpec("core"),) * n_in,
                      out_specs=(PartitionSpec("core"),) * len(out_names),
                      check_rep=False),
            keep_unused=True)

    def run(self, in_maps):
        concat_in = [np.concatenate([m[n] for m in in_maps], axis=0)
                     for n in self.in_names]
        concat_zeros = [np.zeros((8 * a.shape[0], *a.shape[1:]), a.dtype)
                        for a in self.out_avals]
        outs = self.fn(*concat_in, *concat_zeros)
        outs = [np.asarray(o) for o in outs]
        return [
            {n: outs[i].reshape(8, *self.out_avals[i].shape)[c]
             for i, n in enumerate(self.out_names)}
            for c in range(8)
        ]


def compile_with_reps(reps):
    return _Compiled(reps)


def _dec(v):
    """bf16 hi/lo split: v ~= hi + lo exactly to ~2^-18 rel."""
    h = v.astype(BF16)
    l = (v - h.astype(np.float32)).astype(BF16)
    return h, l


def _dec3(v):
    """bf16 hi/mid/lo split: v ~= h + m + l exactly to ~2^-27 rel."""
    h = v.astype(BF16)
    r = v - h.astype(np.float32)
    m = r.astype(BF16)
    l = (r - m.astype(np.float32)).astype(BF16)
    return h, m, l


def make_in_maps(predicted, target, predicted_features, target_features,
                 chunk1, chunk2):
    """Host-side packing of the full inputs into 8 per-core input maps."""
    predicted = np.ascontiguousarray(predicted, dtype=np.float32)
    target = np.ascontiguousarray(target, dtype=np.float32)
    chunk1 = np.asarray(chunk1, np.float32)
    chunk2 = np.asarray(chunk2, np.float32)

    # boundary c2 side is shared by all cores (3-term split: min_d^2 ~ 1e-5
    # is below the 2-term decomposition noise)
    ch, cm, cl = _dec3(chunk2)                         # [4096, 3]
    c22 = (chunk2.astype(np.float64) ** 2).sum(1).astype(np.float32)
    c2h, c2m, c2l = _dec3(c22)
    bmov = np.zeros((21, HALF), BF16)
    bmov[0:3] = ch.T
    bmov[3:6] = cm.T
    bmov[6:9] = ch.T
    bmov[9:12] = cm.T
    bmov[12:15] = cl.T
    bmov[15:18] = ch.T
    bmov[18] = -c2h
    bmov[19] = -c2m
    bmov[20] = -c2l

    in_maps = []
    for core in range(8):
        b, h = core // 2, core % 2
        X = predicted[b]
        xp = np.concatenate([X[h * HALF:(h + 1) * HALF],
                             X[(1 - h) * HALF:(2 - h) * HALF]], axis=0)
        xh, xl = _dec(xp)                              # [8192, 3]
        x2 = (xp.astype(np.float64) ** 2).sum(1).astype(np.float32)
        x2h, x2l = _dec(x2)

        sjt = np.zeros((13, N), BF16)
        sjt[0:3] = xh.T
        sjt[3:6] = xh.T
        sjt[6:9] = xl.T
        sjt[9] = -x2h
        sjt[10] = -x2l
        sjt[11] = 1.0
        sjt[12] = 1.0

        xh_o = xh[0:HALF].astype(np.float32)
        xl_o = xl[0:HALF].astype(np.float32)
        mbt = np.zeros((13, HALF), BF16)
        mbt[0:3] = (2.0 * xh_o).astype(BF16).T
        mbt[3:6] = (2.0 * xl_o).astype(BF16).T
        mbt[6:9] = (2.0 * xh_o).astype(BF16).T
        mbt[9] = 1.0
        mbt[10] = 1.0
        # rows 11, 12 (tau) are filled on device

        # mask-mm j-table: j = w*128 + p
        wt = np.zeros((128, NJT, 5), np.float16)
        xr = xp.reshape(NJT, 128, 3).transpose(1, 0, 2)  # [128, 64, 3]
        wt[:, :, 0:3] = xr.astype(np.float16)
        wt[:, :, 3] = x2.reshape(NJT, 128).T.astype(np.float16)
        wt[:, :, 4] = 1.0
        wtab = np.ascontiguousarray(wt.reshape(128, NJT * 5))

        prow_ = np.ascontiguousarray(
            predicted[b, h * HALF:(h + 1) * HALF].reshape(128, 96))
        trow_ = np.ascontiguousarray(
            target[b, h * HALF:(h + 1) * HALF].reshape(128, 96))
        if h == 0:
            pf = np.ascontiguousarray(
                predicted_features[b].reshape(128, 8).astype(np.float32))
            tf = np.ascontiguousarray(
                target_features[b].reshape(128, 8).astype(np.float32))
        else:
            pf = np.zeros((128, 8), np.float32)
            tf = np.zeros((128, 8), np.float32)

        c1s = chunk1[core * 512:(core + 1) * 512]      # [512, 3]
        c1h, c1m, c1l = _dec3(c1s)
        t2 = lambda a: (2.0 * a.astype(np.float32)).astype(BF16).T
        bsta = np.zeros((21, 512), BF16)
        bsta[0:3] = t2(c1h)
        bsta[3:6] = t2(c1h)
        bsta[6:9] = t2(c1m)
        bsta[9:12] = t2(c1m)
        bsta[12:15] = t2(c1h)
        bsta[15:18] = t2(c1l)
        bsta[18] = 1.0
        bsta[19] = 1.0
        bsta[20] = 1.0
        c1r2_ = np.ascontiguousarray(
            (c1s.astype(np.float64) ** 2).sum(1)
            .reshape(4, 128).T.astype(np.float32))

        in_maps.append({
            "sjt": sjt, "mbt": mbt, "wtab": wtab,
            "prow": prow_, "trow": trow_, "pfeat": pf, "tfeat": tf,
            "bsta": bsta, "bmov": bmov, "c1r2": c1r2_,
        })
    return in_maps


def combine(results):
    """Host-side unshard: sum per-core partials -> the 5 output scalars."""
    rec = per = cont = bs = bc = 0.0
    for r in results:
        o = r["out"].astype(np.float64)
        cont += o[:, 0].sum()
        rec += o[:, 1].sum()
        per += o[:, 2].sum()
        bs += o[:, 3].sum()
        bc += o[:, 4].sum()
    recon = rec / (B * N * 3)
    percep = per / (B * 1024)
    cont = cont / (B * N * 8)
    bcr = np.round(bc)
    bnd = bs / max(bcr, 1.0) if bcr > 0 else 0.0
    total = 1.0 * recon + 0.5 * percep + 0.5 * cont + 1.0 * bnd
    return np.array([recon, percep, cont, bnd, total], dtype=np.float32)


def kernel(**inputs):
    global _COMPILED
    if _COMPILED is None:
        _COMPILED = _Compiled()
    in_maps = make_in_maps(**{k: np.asarray(v) for k, v in inputs.items()})
    results = _COMPILED.run(in_maps)
    return combine(results)


if __name__ == "__main__":
    d = np.load("/root/problem/inputs_cache.npz")
    got = kernel(**{k: d[k] for k in d.files})
    exp = np.load("/root/problem/expected_cache.npy")
    print("got:", got)
    print("exp:", exp)
    print("rel:", np.abs(got - exp) / np.maximum(np.abs(exp), 1e-12))
